# revision 23
# baseline (speedup 1.0000x reference)
"""Multi-Head Latent Attention (MLA) Bass kernel for Trainium2, 8 NeuronCores.

Problem: B=2, S=2048, D=2048, H=16, D_NOPE=128, D_ROPE=64, D_V=128, R_Q=1536, R_KV=512.

Sharding: core c = b*4 + g handles batch b, head group g (heads 4g..4g+3).
Compress (x -> cq/ckv/krope latents) is sequence-sharded across the 4 cores of a
batch group; latents are AllGathered within the group (with ssq rows for remote
RMSNorm). Decompress, attention and out-proj are head-sharded; each core emits a
partial y^T (bf16) which the host sums.

Schedule: the q-latent AllGather is split in two halves (rows 0-767 fire after
the first cq half, rows 768-1535+ssq after the second), so kv-decompress and the
first q-decompress accumulation stage stream underneath the collectives.

Key algebraic simplifications (exact):
- RoPE rotations cancel in q.k (per-head angles, same rotation on q/k), skipped.
- RMSNorm folded post-decompress via broadcast rstd tiles; norm weights and the
  V-scale 1/sqrt(H*D_V) folded into decompress weights on the host.
- Softmax without max subtraction; probs = exp(s)*tri_mask on diagonal blocks;
  l = ones-matmul column sums; out = (V^T P) * bcast(1/l).
- Causal diagonal trimming: for key-tile t only queries q >= 128*t are computed.
"""
import sys
sys.path.insert(0, '/opt/trn_rl_repo')

import numpy as np
import ml_dtypes
from contextlib import ExitStack

from concourse import bacc, tile
import concourse.mybir as mybir
from concourse.bass_utils import run_bass_kernel_spmd

f32 = mybir.dt.float32
f32r = mybir.dt.float32r
bf16 = mybir.dt.bfloat16

B, S, D = 2, 2048, 2048
H, DN, DR, DV = 16, 128, 64, 128
RQ, RKV = 1536, 512
EPS = 1e-5
HG = 4                      # heads per group
SC = 512                    # S-chunk width
NC_ = 8                     # cores
ATTN_SCALE = float(1.0 / np.sqrt(DN + DR))
Act = mybir.ActivationFunctionType

MW = HG * (DN + DR)         # 768: merged q-decompress M width

_CACHED_NC = None


def _build():
    nc = bacc.Bacc("TRN2", target_bir_lowering=False, debug=False, num_devices=NC_)

    xs = nc.declare_dram_parameter("xs", [D, SC], bf16, isOutput=False)
    w_cq = nc.declare_dram_parameter("w_cq", [D, RQ], bf16, isOutput=False)
    w_ckv = nc.declare_dram_parameter("w_ckv", [D, RKV], bf16, isOutput=False)
    w_kr = nc.declare_dram_parameter("w_kr", [D, DR], bf16, isOutput=False)
    # merged q-decompress weights: M-tiles 0-3 = qn head m, 4-5 = qr pairs
    w_dq = nc.declare_dram_parameter("w_dq", [RQ, MW], bf16, isOutput=False)
    w_dk = nc.declare_dram_parameter("w_dk", [RKV, HG * DN], bf16, isOutput=False)
    w_dv = nc.declare_dram_parameter("w_dv", [RKV, HG * DV], bf16, isOutput=False)
    w_proj = nc.declare_dram_parameter("w_proj", [HG * DV, D], bf16, isOutput=False)
    tri_in = nc.declare_dram_parameter("tri", [128, 128], bf16, isOutput=False)
    ones_r_in = nc.declare_dram_parameter("ones_r", [128, 128], f32r, isOutput=False)
    ones_b_in = nc.declare_dram_parameter("ones_b", [128, 1], bf16, isOutput=False)
    yT = nc.declare_dram_parameter("yT", [D, S], bf16, isOutput=True)

    with tile.TileContext(nc) as tc, ExitStack() as ctx:
        keep = ctx.enter_context(tc.tile_pool(name="keep", bufs=1))
        dram = ctx.enter_context(tc.tile_pool(name="dram", bufs=1, space="DRAM"))

        ones_r = keep.tile([128, 128], f32r)
        nc.sync.dma_start(ones_r[:], ones_r_in[:])
        ones_b = keep.tile([128, 1], bf16)
        nc.sync.dma_start(ones_b[:], ones_b_in[:])
        tri = keep.tile([128, 128], bf16)
        nc.sync.dma_start(tri[:], tri_in[:])
        wdq_sb = keep.tile([128, 12 * MW], bf16)   # r-tile r at cols r*MW

        # kv latents: nkv 0-511 | krope 512-575 | ssq_kv hi 576 lo 577
        lat_kv_in = dram.tile([RKV + DR + 2, SC], bf16)
        lat_kv = dram.tile([4, RKV + DR + 2, SC], bf16)
        # q latents in two halves: A rows 0-767; B rows 768-1535 + ssq hi/lo
        lat_qa_in = dram.tile([768, SC], bf16)
        lat_qa = dram.tile([4, 768, SC], bf16)
        lat_qb_in = dram.tile([768 + 2, SC], bf16)
        lat_qb = dram.tile([4, 768 + 2, SC], bf16)

        # ============ Phase C: compress own S-shard (kv first, then q) ============
        with ExitStack() as c_ctx:
            wstream = c_ctx.enter_context(tc.tile_pool(name="wstream", bufs=5))
            cout = c_ctx.enter_context(tc.tile_pool(name="cout", bufs=4))
            cps = c_ctx.enter_context(tc.tile_pool(name="cps", bufs=1, space="PSUM"))

            xs_sb = cout.tile([128, 16 * SC], bf16, tag="xs", bufs=1)

            def ssq_hilo(psum_row, dest_dram, row_off):
                """Split fp32 psum row into bf16 hi/lo rows and DMA to dest."""
                full = cout.tile([1, SC], f32, tag="ssqf")
                nc.vector.tensor_copy(full[:], psum_row[:])
                hi = cout.tile([1, SC], bf16, tag="ssqh")
                nc.vector.tensor_copy(hi[:], full[:])
                lo = cout.tile([1, SC], bf16, tag="ssql")
                nc.vector.tensor_sub(lo[:], full[:], hi[:])
                nc.sync.dma_start(dest_dram[row_off:row_off + 1, :], hi[:])
                nc.sync.dma_start(dest_dram[row_off + 1:row_off + 2, :], lo[:])

            # ---- nkv: 4 r-tiles ----
            psum_ssq_kv = cps.tile([1, SC], f32, tag="ssq_kv")
            psums = [cps.tile([128, SC], f32, tag=f"cqp{i}", name=f"psum_kv{i}") for i in range(4)]
            for d in range(16):
                nc.sync.dma_start(xs_sb[:, d * SC:(d + 1) * SC], xs[d * 128:(d + 1) * 128, :])
                wt = wstream.tile([128, RKV], bf16, tag="wckv")
                nc.sync.dma_start(wt[:], w_ckv[d * 128:(d + 1) * 128, :])
                for i in range(4):
                    nc.tensor.matmul(psums[i][:], wt[:, i * 128:(i + 1) * 128],
                                     xs_sb[:, d * SC:(d + 1) * SC],
                                     start=(d == 0), stop=(d == 15))
            for i in range(4):
                sq = cout.tile([128, SC], bf16, tag="sq")
                nc.scalar.activation(sq[:], psums[i][:], Act.Square)
                ckv = cout.tile([128, SC], bf16, tag="cq")
                nc.vector.tensor_copy(ckv[:], psums[i][:])
                nc.sync.dma_start(lat_kv_in[i * 128:(i + 1) * 128, :], ckv[:])
                nc.tensor.matmul(psum_ssq_kv[:], ones_b[:], sq[:],
                                 start=(i == 0), stop=(i == 3))

            # ---- krope: [64, SC] ----
            psum_kr = cps.tile([64, SC], f32, tag="cqp4")
            for d in range(16):
                wt = wstream.tile([128, DR], bf16, tag="wkr")
                nc.sync.dma_start(wt[:], w_kr[d * 128:(d + 1) * 128, :])
                nc.tensor.matmul(psum_kr[:], wt[:], xs_sb[:, d * SC:(d + 1) * SC],
                                 start=(d == 0), stop=(d == 15))
            krc = cout.tile([64, SC], bf16, tag="cq")
            nc.vector.tensor_copy(krc[:], psum_kr[:])
            nc.sync.dma_start(lat_kv_in[RKV:RKV + DR, :], krc[:])
            ssq_hilo(psum_ssq_kv, lat_kv_in, RKV + DR)

            # ---- AllGather 1 (kv latents) ----
            nc.gpsimd.collective_compute(
                "AllGather", mybir.AluOpType.bypass,
                replica_groups=[[0, 1, 2, 3], [4, 5, 6, 7]],
                ins=[lat_kv_in[:]], outs=[lat_kv[:]],
            )

            # ---- cq: 12 r-tiles in halves of 6 (6 psum banks) ----
            psum_ssq_q = cps.tile([1, SC], f32, tag="ssq_q")
            for half in range(2):
                lat_half_in = lat_qa_in if half == 0 else lat_qb_in
                psums = [cps.tile([128, SC], f32, tag=f"cqp{i}", name=f"psum_cq{i}") for i in range(6)]
                for d in range(16):
                    wt = wstream.tile([128, 6 * 128], bf16, tag="wcq")
                    nc.sync.dma_start(wt[:], w_cq[d * 128:(d + 1) * 128,
                                                  half * 768:(half + 1) * 768])
                    for i in range(6):
                        nc.tensor.matmul(psums[i][:], wt[:, i * 128:(i + 1) * 128],
                                         xs_sb[:, d * SC:(d + 1) * SC],
                                         start=(d == 0), stop=(d == 15))
                for i in range(6):
                    r = half * 6 + i
                    sq = cout.tile([128, SC], bf16, tag="sq")
                    nc.scalar.activation(sq[:], psums[i][:], Act.Square)
                    cq = cout.tile([128, SC], bf16, tag="cq")
                    nc.vector.tensor_copy(cq[:], psums[i][:])
                    nc.sync.dma_start(lat_half_in[i * 128:(i + 1) * 128, :], cq[:])
                    nc.tensor.matmul(psum_ssq_q[:], ones_b[:], sq[:],
                                     start=(r == 0), stop=(r == 11))
                if half == 0:
                    # ---- AllGather 2a (first q-latent half) ----
                    nc.gpsimd.collective_compute(
                        "AllGather", mybir.AluOpType.bypass,
                        replica_groups=[[0, 1, 2, 3], [4, 5, 6, 7]],
                        ins=[lat_qa_in[:]], outs=[lat_qa[:]],
                    )
            ssq_hilo(psum_ssq_q, lat_qb_in, 768)

            # prefetch q-decompress weights (2.4MB); needed only by Dq stage A
            for r in range(12):
                nc.sync.dma_start(wdq_sb[:, r * MW:(r + 1) * MW],
                                  w_dq[r * 128:(r + 1) * 128, :])

            # ---- AllGather 2b (second q-latent half + ssq) ----
            nc.gpsimd.collective_compute(
                "AllGather", mybir.AluOpType.bypass,
                replica_groups=[[0, 1, 2, 3], [4, 5, 6, 7]],
                ins=[lat_qb_in[:]], outs=[lat_qb[:]],
            )

        def rstd_bcast(pool, psum_pool, hi_src, lo_src, rr, cname, want_cols=False):
            """rstd broadcast tile [128,SC] f32 (and optional [128,4] col tile)."""
            hi = pool.tile([1, SC], bf16, tag="ssqh", name=f"ssqh_{cname}")
            lo = pool.tile([1, SC], bf16, tag="ssql", name=f"ssql_{cname}")
            nc.sync.dma_start(hi[:], hi_src)
            nc.sync.dma_start(lo[:], lo_src)
            ssq_t = pool.tile([1, SC], f32, tag="ssq", name=f"ssq_{cname}")
            nc.vector.tensor_add(ssq_t[:], hi[:], lo[:])
            eps_t = pool.tile([1, 1], f32, tag="eps", name=f"eps_{cname}")
            nc.vector.memset(eps_t[:], EPS)
            std = pool.tile([1, SC], f32, tag="std", name=f"std_{cname}")
            nc.scalar.activation(std[:], ssq_t[:], Act.Sqrt, scale=1.0 / rr, bias=eps_t[:])
            rstd = pool.tile([1, SC], f32, tag="rstd", name=f"rstd_{cname}")
            scr = pool.tile([1, SC], f32, tag="scr", name=f"scr_{cname}")
            nc.vector.reciprocal_approx_accurate(rstd[:], std[:], scr[:])
            bt = pool.tile([128, SC], f32, tag=f"bc_{cname}", name=f"bt_{cname}")
            nc.gpsimd.partition_broadcast(bt[:], rstd[:])
            ct = None
            if want_cols:
                # transpose row->col via tiny plain-f32 matmuls (f32r fails at N=1)
                onet = pool.tile([1, 1], f32, tag="onet", name=f"onet_{cname}")
                nc.vector.memset(onet[:], 1.0)
                pcol = psum_pool.tile([128, 4], f32, tag="col", bufs=1, name=f"pcol_{cname}")
                for i in range(4):
                    nc.tensor.matmul(pcol[:, i:i + 1],
                                     rstd[0:1, i * 128:(i + 1) * 128],
                                     onet[:], start=True, stop=True)
                ct = pool.tile([128, 4], f32, tag=f"col_{cname}", name=f"colt_{cname}")
                nc.vector.tensor_copy(ct[:], pcol[:])
            return bt, ct

        # ============ Phase Dkv: decompress k_nope and v (after AllGather 1) =====
        kv_pool = ctx.enter_context(tc.tile_pool(name="kvp", bufs=1))
        k_sb = [kv_pool.tile([128, S], bf16, tag=f"k{h}", name=f"k_sb{h}") for h in range(HG)]
        v_sb = kv_pool.tile([128, 16 * SC], bf16, tag="v")
        # krope duplicated on partitions 0-63 / 64-127 (odd heads at row offset 64)
        krope_sb = kv_pool.tile([128, S], bf16, tag="krope")
        wdk = ctx.enter_context(tc.tile_pool(name="wdk", bufs=1))
        nkvp = ctx.enter_context(tc.tile_pool(name="nkvp", bufs=1))
        with ExitStack() as dk_ctx:
            kps = dk_ctx.enter_context(tc.tile_pool(name="kps", bufs=1, space="PSUM"))

            pairs = [rstd_bcast(nkvp, kps, lat_kv[c, RKV + DR:RKV + DR + 1, :],
                                lat_kv[c, RKV + DR + 1:RKV + DR + 2, :], RKV,
                                f"kv{c}", want_cols=True) for c in range(4)]
            bcast_kv = [p[0] for p in pairs]
            rstdkv_col = [p[1] for p in pairs]

            wdk_sb = wdk.tile([128, 4 * HG * DN], bf16)    # r-tile r at cols r*512
            wdv_sb = wdk.tile([128, 4 * HG * DV], bf16)
            for r in range(4):
                nc.sync.dma_start(wdk_sb[:, r * 512:(r + 1) * 512], w_dk[r * 128:(r + 1) * 128, :])
                nc.sync.dma_start(wdv_sb[:, r * 512:(r + 1) * 512], w_dv[r * 128:(r + 1) * 128, :])
            nkv_sb = nkvp.tile([128, 4 * 4 * SC], bf16)    # (r, c) at cols (r*4+c)*SC
            for r in range(4):
                for c in range(4):
                    nc.sync.dma_start(nkv_sb[:, (r * 4 + c) * SC:(r * 4 + c + 1) * SC],
                                      lat_kv[c, r * 128:(r + 1) * 128, :])
            for c in range(4):
                nc.sync.dma_start(krope_sb[0:64, c * SC:(c + 1) * SC],
                                  lat_kv[c, RKV:RKV + DR, :])
                nc.sync.dma_start(krope_sb[64:128, c * SC:(c + 1) * SC],
                                  lat_kv[c, RKV:RKV + DR, :])

            # k_nope
            for h in range(HG):
                pk = [kps.tile([128, SC], f32, tag=f"k{c}", name=f"pk{c}") for c in range(4)]
                for r in range(4):
                    for c in range(4):
                        nc.tensor.matmul(pk[c][:],
                                         wdk_sb[:, r * 512 + h * DN:r * 512 + (h + 1) * DN],
                                         nkv_sb[:, (r * 4 + c) * SC:(r * 4 + c + 1) * SC],
                                         start=(r == 0), stop=(r == 3))
                for c in range(4):
                    nc.vector.tensor_mul(k_sb[h][:, c * SC:(c + 1) * SC], pk[c][:], bcast_kv[c][:])

        # ============ Phase Dq: q decompress, 2-stage accumulation per chunk =====
        # (v decompress is emitted between the stages: it depends only on
        # AllGather 1, so it feeds the PE while stage B waits for AllGather 2b)
        qp = ctx.enter_context(tc.tile_pool(name="qp", bufs=1))
        qn_sb = [qp.tile([128, S], bf16, tag=f"qn{h}", name=f"qn_sb{h}") for h in range(HG)]
        # qr pairs packed: qr2_sb[p] rows 0-63 = qr of head 2p, 64-127 = head 2p+1
        qr2_sb = [qp.tile([128, S], bf16, tag=f"qr{p}", name=f"qr2_sb{p}") for p in range(2)]
        with ExitStack() as dq_ctx:
            nqp = dq_ctx.enter_context(tc.tile_pool(name="nqp", bufs=2))
            qps = dq_ctx.enter_context(tc.tile_pool(name="qps", bufs=1, space="PSUM"))

            def qdest(mt, c):
                if mt < 4:
                    return qn_sb[mt][:, c * SC:(c + 1) * SC]
                return qr2_sb[mt - 4][:, c * SC:(c + 1) * SC]

            # stage A for ALL chunks first (needs only AllGather 2a), spilled
            # into qn/qr2 as bf16 partials, so the PE never blocks on 2b
            for c in range(4):
                nq_half = nqp.tile([128, 6 * SC], bf16, tag="nqa", name=f"nqa{c}")
                for i in range(6):
                    nc.sync.dma_start(nq_half[:, i * SC:(i + 1) * SC],
                                      lat_qa[c, i * 128:(i + 1) * 128, :])
                for mt in range(6):
                    pq = qps.tile([128, SC], f32, tag=f"dq{mt}", name=f"pqa{c}_{mt}")
                    for i in range(6):
                        nc.tensor.matmul(pq[:],
                                         wdq_sb[:, i * MW + mt * 128:i * MW + (mt + 1) * 128],
                                         nq_half[:, i * SC:(i + 1) * SC],
                                         start=(i == 0), stop=(i == 5))
                    nc.vector.tensor_copy(qdest(mt, c), pq[:])

            # v (row-major, all heads at once), scaled by rstd_kv rows
            for t in range(16):
                c, i = divmod(t, 4)
                pv = qps.tile([128, SC], f32, tag="vps", bufs=2)
                for r in range(4):
                    nc.tensor.matmul(pv[:],
                                     nkv_sb[:, (r * 4 + c) * SC + i * 128:(r * 4 + c) * SC + (i + 1) * 128],
                                     wdv_sb[:, r * 512:(r + 1) * 512],
                                     start=(r == 0), stop=(r == 3))
                nc.vector.tensor_scalar_mul(v_sb[:, t * SC:(t + 1) * SC], pv[:],
                                            rstdkv_col[c][:, i:i + 1])

            # stage B accumulates r-tiles 6-11 and finalizes with rstd
            for c in range(4):
                nq_half = nqp.tile([128, 6 * SC], bf16, tag="nqb", name=f"nqb{c}")
                for i in range(6):
                    nc.sync.dma_start(nq_half[:, i * SC:(i + 1) * SC],
                                      lat_qb[c, i * 128:(i + 1) * 128, :])
                bt, _ = rstd_bcast(nqp, qps, lat_qb[c, 768:769, :], lat_qb[c, 769:770, :],
                                   RQ, f"q{c}")
                for mt in range(6):
                    pq = qps.tile([128, SC], f32, tag=f"dq{mt}", name=f"pqb{c}_{mt}")
                    for i in range(6):
                        r = 6 + i
                        nc.tensor.matmul(pq[:],
                                         wdq_sb[:, r * MW + mt * 128:r * MW + (mt + 1) * 128],
                                         nq_half[:, i * SC:(i + 1) * SC],
                                         start=(i == 0), stop=(i == 5))
                    qsum = nqp.tile([128, SC], f32, tag="qsum", name=f"qsum{c}_{mt}")
                    nc.vector.tensor_add(qsum[:], pq[:], qdest(mt, c))
                    nc.vector.tensor_mul(qdest(mt, c), qsum[:], bt[:])

        # ============ Phase A: attention + per-chunk projection ============
        with ExitStack() as a_ctx:
            wp = a_ctx.enter_context(tc.tile_pool(name="wp", bufs=1))
            probs_pool = a_ctx.enter_context(tc.tile_pool(name="probs", bufs=10))
            awork = a_ctx.enter_context(tc.tile_pool(name="awork", bufs=2))
            aps = a_ctx.enter_context(tc.tile_pool(name="aps", bufs=1, space="PSUM"))

            wproj_sb = wp.tile([128, HG * D], bf16)   # head h rows at cols h*D
            for h in range(HG):
                nc.sync.dma_start(wproj_sb[:, h * D:(h + 1) * D],
                                  w_proj[h * 128:(h + 1) * 128, :])

            for j in range(4):
                T = 4 * (j + 1)
                outc = [awork.tile([128, SC], bf16, tag=f"oc{h}", name=f"outc{h}_{j}", bufs=2)
                        for h in range(HG)]
                for h in range(HG):
                    p2, e = divmod(h, 2)     # qr pair index, row offset selector
                    ro = 64 * e
                    psum_l = aps.tile([1, SC], f32, tag="l", bufs=1)
                    psum_o = aps.tile([128, SC], f32, tag="o", bufs=2)
                    state = {"started": False}

                    def consume(grp, last):
                        st = not state["started"]
                        state["started"] = True
                        if grp[0] == "pair":
                            # one l matmul over the DVE pair-sum of two full tiles
                            _, t0, pt0, pt1, psum2 = grp
                            nc.tensor.matmul(psum_l[:], ones_b[:], psum2[:],
                                             start=st, stop=last)
                            nc.tensor.matmul(psum_o[:],
                                             v_sb[:, t0 * SC + h * DV:t0 * SC + (h + 1) * DV],
                                             pt0[:], start=st, stop=False)
                            nc.tensor.matmul(psum_o[:],
                                             v_sb[:, (t0 + 1) * SC + h * DV:(t0 + 1) * SC + (h + 1) * DV],
                                             pt1[:], start=False, stop=last)
                        else:
                            _, t, pt, off = grp
                            nc.tensor.matmul(psum_l[:, off:], ones_b[:], pt[:, off:],
                                             start=st, stop=last)
                            nc.tensor.matmul(psum_o[:, off:],
                                             v_sb[:, t * SC + h * DV:t * SC + (h + 1) * DV],
                                             pt[:, off:], start=st, stop=last)

                    groups = []
                    pbuf = None
                    for t in range(T):
                        # causal trim: queries < 128*t never see key-tile t
                        off = max(0, 128 * t - 512 * j)
                        psum_s = aps.tile([128, SC], f32, tag="s", bufs=3)
                        nc.tensor.matmul(psum_s[:, off:],
                                         k_sb[h][:, t * 128:(t + 1) * 128],
                                         qn_sb[h][:, j * SC + off:(j + 1) * SC],
                                         start=True, stop=False)
                        nc.tensor.matmul(psum_s[:, off:],
                                         krope_sb[ro:ro + 64, t * 128:(t + 1) * 128],
                                         qr2_sb[p2][ro:ro + 64, j * SC + off:(j + 1) * SC],
                                         start=False, stop=True)
                        pt = probs_pool.tile([128, SC], bf16, tag="p")
                        nc.scalar.activation(pt[:, off:], psum_s[:, off:],
                                             Act.Exp, scale=ATTN_SCALE)
                        if t >= 4 * j:
                            # mask the leading [128,128] triangle block
                            nc.vector.tensor_mul(pt[:, off:off + 128],
                                                 pt[:, off:off + 128], tri[:])
                            groups.append(("single", t, pt, off))
                        elif pbuf is None:
                            pbuf = (t, pt)
                        else:
                            t0, pt0 = pbuf
                            pbuf = None
                            psum2 = probs_pool.tile([128, SC], bf16, tag="ps2")
                            nc.vector.tensor_add(psum2[:], pt0[:], pt[:])
                            groups.append(("pair", t0, pt0, pt, psum2))
                        while len(groups) > 3:
                            consume(groups.pop(0), False)
                    for idx, grp in enumerate(groups):
                        consume(grp, idx == len(groups) - 1)
                    l_sb = awork.tile([1, SC], f32, tag="l")
                    nc.vector.tensor_copy(l_sb[:], psum_l[:])
                    rinv = awork.tile([1, SC], f32, tag="rinv")
                    scr = awork.tile([1, SC], f32, tag="scr")
                    nc.vector.reciprocal_approx_accurate(rinv[:], l_sb[:], scr[:])
                    binv = awork.tile([128, SC], f32, tag="binv")
                    nc.gpsimd.partition_broadcast(binv[:], rinv[:])
                    nc.vector.tensor_mul(outc[h][:], psum_o[:], binv[:])

                # projection for chunk j
                for dout in range(16):
                    ppj = aps.tile([128, SC], f32, tag="pj", bufs=2)
                    for h in range(HG):
                        nc.tensor.matmul(ppj[:],
                                         wproj_sb[:, h * D + dout * 128:h * D + (dout + 1) * 128],
                                         outc[h][:], start=(h == 0), stop=(h == HG - 1))
                    y_sb = awork.tile([128, SC], bf16, tag="y", bufs=3)
                    nc.scalar.copy(y_sb[:], ppj[:])
                    nc.sync.dma_start(yT[dout * 128:(dout + 1) * 128, j * SC:(j + 1) * SC], y_sb[:])

    nc.compile()
    return nc


def _get_nc():
    global _CACHED_NC
    if _CACHED_NC is None:
        _CACHED_NC = _build()
    return _CACHED_NC


def kernel(x, mask, freqs_cos, freqs_sin, w_cq, q_norm_w, w_dq_nope, w_dq_rope,
           w_ckv, kv_norm_w, w_dk_nope, w_dv, w_k_rope, w_proj, **_unused):
    x = np.asarray(x, np.float32)
    w_cq = np.asarray(w_cq, np.float32)
    w_ckv = np.asarray(w_ckv, np.float32)
    w_k_rope = np.asarray(w_k_rope, np.float32)
    q_norm_w = np.asarray(q_norm_w, np.float32)
    kv_norm_w = np.asarray(kv_norm_w, np.float32)

    # fold norm weights / v-scale into decompress weights
    w_dqn = q_norm_w[:, None] * np.asarray(w_dq_nope, np.float32)
    w_dqr = q_norm_w[:, None] * np.asarray(w_dq_rope, np.float32)
    w_dk = kv_norm_w[:, None] * np.asarray(w_dk_nope, np.float32)
    w_dv_f = kv_norm_w[:, None] * np.asarray(w_dv, np.float32) * np.float32(1.0 / np.sqrt(H * DV))
    w_proj = np.asarray(w_proj, np.float32)

    tri_np = (np.arange(128)[:, None] <= np.arange(128)[None, :]).astype(np.float32)
    tri_np = tri_np.astype(ml_dtypes.bfloat16)
    ones_r = np.ones((128, 128), np.float32)
    ones_b = np.ones((128, 1), np.float32).astype(ml_dtypes.bfloat16)

    xT = [np.ascontiguousarray(x[b].T) for b in range(B)]
    w_cq_b = w_cq.astype(ml_dtypes.bfloat16)
    w_ckv_b = w_ckv.astype(ml_dtypes.bfloat16)
    w_kr_b = w_k_rope.astype(ml_dtypes.bfloat16)

    in_maps = []
    for c in range(NC_):
        b, g = divmod(c, 4)
        hs = g * HG                     # first head of group
        # merged q-decompress weights: 4 nope heads then 2 rope pairs
        wq_cols = [w_dqn[:, (hs + i) * DN:(hs + i + 1) * DN] for i in range(HG)]
        wq_cols += [np.concatenate([w_dqr[:, (hs + 2 * p) * DR:(hs + 2 * p + 1) * DR],
                                    w_dqr[:, (hs + 2 * p + 1) * DR:(hs + 2 * p + 2) * DR]],
                                   axis=1) for p in range(2)]
        w_dq_merged = np.concatenate(wq_cols, axis=1)
        in_maps.append({
            "xs": np.ascontiguousarray(xT[b][:, g * SC:(g + 1) * SC]).astype(ml_dtypes.bfloat16),
            "w_cq": w_cq_b,
            "w_ckv": w_ckv_b,
            "w_kr": w_kr_b,
            "w_dq": np.ascontiguousarray(w_dq_merged).astype(ml_dtypes.bfloat16),
            "w_dk": np.ascontiguousarray(w_dk[:, hs * DN:(hs + HG) * DN]).astype(ml_dtypes.bfloat16),
            "w_dv": np.ascontiguousarray(w_dv_f[:, hs * DV:(hs + HG) * DV]).astype(ml_dtypes.bfloat16),
            "w_proj": np.ascontiguousarray(w_proj[hs * DV:(hs + HG) * DV, :]).astype(ml_dtypes.bfloat16),
            "tri": tri_np,
            "ones_r": ones_r,
            "ones_b": ones_b,
        })

    nc = _get_nc()
    res = run_bass_kernel_spmd(nc, in_maps, list(range(NC_)))

    out = np.zeros((B, S, D), np.float32)
    for c in range(NC_):
        b = c // 4
        out[b] += res.results[c]["yT"].T.astype(np.float32)
    return out
